# revision 26
# baseline (speedup 1.0000x reference)
"""Segment-sum (AggrSum) kernel for 8 Trainium2 NeuronCores.

Math: out[v, :] = sum_{n: X_neis[n] == v} H[n, :]   (H [N, D], out [V, D])

Strategy (V-sharding with host-side bucketing as the sharding step):
  - Sort edge ids by target vocab index; group edges by 128-row vocab tile.
  - Partition the 64 vocab tiles into 8 balanced groups of 8 (one per
    core), ordered inside each group so that packed prefix drift stays
    within +-128 rows of 512*vt ("mode B"). Each core then reads an
    exactly-packed, zero-padding-free edge stream; every vocab tile's
    edges are covered by a fixed window of 6 physical 128-row tiles at
    static offsets, and the one-hot masks zero out foreign rows.
  - H rows are uploaded as an exact bf16 hi/lo split; each (vt, k)
    window tile needs two bf16 matmuls (hi, lo) accumulating into the
    same [128, 256] fp32 PSUM tile. One DVE is_equal per vocab tile
    builds all six masks at once. Each core writes its own eight output
    tiles; the host scatters them into the full [V, D].
  - If the drift bound fails (pathological inputs), fall back to
    "mode A": pad every vocab tile to K*128 rows (no windows).
"""

import numpy as np

import concourse.bacc as bacc
import concourse.mybir as mybir
import concourse.tile as tile
from concourse.bass_utils import run_bass_kernel_spmd

N, D, V = 32768, 256, 8192
NCORES = 8
P = 128
VT_PER_CORE = V // P // NCORES  # 8 vocab tiles of 128 per core
NVT = V // P  # 64 global vocab tiles
KB = 6  # window tiles per vocab tile in mode B
NTILES_B = 4 * (VT_PER_CORE - 1) - 1 + KB  # 33 physical tiles per core
W = 2 * D  # hi|lo concatenated row width

TRACE = False
LAST_EXEC_NS = None
LAST_RESULTS = None

_PROGRAM_CACHE: dict = {}


def _win_lo(vt: int) -> int:
    """First physical tile of vocab tile vt's window (mode B)."""
    return 0 if vt == 0 else 4 * vt - 1


def _build_common(nc, tc, pools, K, n_phys_tiles, hs, consts, out, chunk_tiles):
    f32 = mybir.dt.float32
    f16 = mybir.dt.float16
    const_pool, hpool, mpool, opool, psum_pool = pools
    nconst = VT_PER_CORE * K + P
    iota_off = VT_PER_CORE * K

    const_sb = const_pool.tile([P, nconst], f16)
    nc.sync.dma_start(const_sb[:], consts[:])

    # chunked prefetch of the packed H stream
    chunks = []
    t0 = 0
    for ct in chunk_tiles:
        ch = hpool.tile([P, ct * W], f16, name="ch")
        nc.sync.dma_start(ch[:], hs[:, t0 * W : (t0 + ct) * W])
        chunks.append((t0, ct, ch))
        t0 += ct
    assert t0 == n_phys_tiles

    def rhs_slice(t, plane):
        for t0, ct, ch in chunks:
            if t0 <= t < t0 + ct:
                off = (t - t0) * W + plane * D
                return ch[:, off : off + D]
        raise AssertionError(t)

    def emit_mask(vt):
        # all K masks for vocab tile vt in one DVE op:
        # m[p, k, q] = (xrel[p, vt*K+k] == iota[q])
        m = mpool.tile([P, K * P], f16, name="m")
        nc.vector.tensor_tensor(
            out=m[:].rearrange("p (k q) -> p k q", k=K),
            in0=const_sb[:, vt * K : (vt + 1) * K]
            .unsqueeze(2)
            .broadcast_to([P, K, P]),
            in1=const_sb[:, iota_off : iota_off + P]
            .unsqueeze(1)
            .broadcast_to([P, K, P]),
            op=mybir.AluOpType.is_equal,
        )
        return m

    # DVE stream order: a few masks ahead, then copies interleaved with the
    # remaining masks so output DMAs start streaming early
    AHEAD = 3
    ms = [emit_mask(vt) for vt in range(min(AHEAD, VT_PER_CORE))]

    for vt in range(VT_PER_CORE):
        m = ms[vt]
        # hi and lo planes accumulate into the same [P, D] psum
        ps = psum_pool.tile([P, D], f32, name="ps")
        for k in range(K):
            t = vt * K + k if n_phys_tiles == VT_PER_CORE * K else _win_lo(vt) + k
            for plane in range(2):
                nc.tensor.matmul(
                    out=ps[:],
                    lhsT=m[:, k * P : (k + 1) * P],
                    rhs=rhs_slice(t, plane),
                    start=(k == 0 and plane == 0),
                    stop=(k == K - 1 and plane == 1),
                )
        if vt + AHEAD < VT_PER_CORE:
            ms.append(emit_mask(vt + AHEAD))
        ot = opool.tile([P, D], f32, name="ot")
        nc.vector.tensor_copy(ot[:], ps[:])
        nc.scalar.dma_start(out[vt * P : (vt + 1) * P, :], ot[:])


def _build_program(mode, K):
    """mode 'B': exact-packed windows (K=KB); mode 'A': padded (K tiles/vt)."""
    f32 = mybir.dt.float32
    f16 = mybir.dt.float16
    if mode == "B":
        n_phys = NTILES_B
        chunk_tiles = [2, 4, 6, 7, 7, 4, 3]
    else:
        n_phys = VT_PER_CORE * K
        nt = n_phys
        chunk_tiles = []
        while nt > 0:
            chunk_tiles.append(min(7, nt))
            nt -= min(7, nt)
    nconst = VT_PER_CORE * K + P

    nc = bacc.Bacc("TRN2", target_bir_lowering=False)
    hs = nc.dram_tensor("hs", [P, n_phys * W], f16, kind="ExternalInput")
    consts = nc.dram_tensor("consts", [P, nconst], f16, kind="ExternalInput")
    out = nc.dram_tensor("out", [VT_PER_CORE * P, D], f32, kind="ExternalOutput")

    with tile.TileContext(nc) as tc:
        with (
            tc.tile_pool(name="const", bufs=1) as const_pool,
            tc.tile_pool(name="h", bufs=min(len(chunk_tiles), 16)) as hpool,
            tc.tile_pool(name="m", bufs=VT_PER_CORE) as mpool,
            tc.tile_pool(name="o", bufs=4) as opool,
            tc.tile_pool(name="psum", bufs=VT_PER_CORE, space="PSUM") as psum_pool,
        ):
            _build_common(
                nc,
                tc,
                (const_pool, hpool, mpool, opool, psum_pool),
                K,
                n_phys,
                hs,
                consts,
                out,
                chunk_tiles,
            )
    nc.finalize()
    return nc


def _partition_tiles(counts):
    """Partition the 64 vocab tiles into 8 groups of 8, ordered so packed
    prefix drift stays in [-128, 128]. Returns groups (list of lists of
    global tile ids) or None if the bound fails."""
    order = np.argsort(counts)[::-1]  # descending by count
    # snake-deal into 8 groups to balance totals
    groups = [[] for _ in range(NCORES)]
    for i, g in enumerate(order):
        rnd, pos = divmod(i, NCORES)
        c = pos if rnd % 2 == 0 else NCORES - 1 - pos
        groups[c].append(int(g))
    final = []
    for c in range(NCORES):
        tiles = sorted(groups[c], key=lambda g: -counts[g])
        # alternate large/small: c0, c7, c1, c6, ...
        seq = []
        i, j = 0, len(tiles) - 1
        while i <= j:
            seq.append(tiles[i])
            if i != j:
                seq.append(tiles[j])
            i += 1
            j -= 1
        # verify drift bound
        run = 0
        for k, g in enumerate(seq):
            drift = run - 512 * k
            if not (-128 <= drift <= 128):
                return None
            run += int(counts[g])
            if k == 0 and run > 768:
                return None
        if not (-128 <= run - 4096 <= 128):
            return None
        final.append(seq)
    return final


def _iota_np():
    return np.tile(np.arange(P, dtype=np.float32), (P, 1))


def _pack_consts(xr, iota_np):
    return np.hstack([xr, iota_np]).astype(np.float16)


def _split_f16(block):
    hi = block.astype(np.float16)
    lo = (block - hi.astype(np.float32)).astype(np.float16)
    return hi, lo


def _tilemajor(hi, lo, ntiles):
    """[ntiles*P, D] hi/lo -> [P, ntiles*W] with per-tile [hi|lo] rows."""
    return (
        np.stack([hi, lo], axis=1)
        .reshape(ntiles, P, W)
        .transpose(1, 0, 2)
        .reshape(P, ntiles * W)
    )


def _shard_mode_b(H, X, order, Xs, counts, starts, groups):
    in_maps = []
    scatter = []  # (core, vt) -> global tile id
    iota_np = _iota_np()
    for c in range(NCORES):
        seq = groups[c]
        rows = np.concatenate([order[starts[g] : starts[g + 1]] for g in seq])
        xval = np.concatenate(
            [Xs[starts[g] : starts[g + 1]] for g in seq]
        ).astype(np.float64)
        n_c = len(rows)
        block = np.zeros((NTILES_B * P, D), dtype=np.float32)
        block[:n_c] = H[rows]
        xpad = np.full(NTILES_B * P, -1000.0, dtype=np.float64)
        xpad[:n_c] = xval
        hi, lo = _split_f16(block)
        hs = _tilemajor(hi, lo, NTILES_B)
        xr = np.full((P, VT_PER_CORE * KB), -1000.0, dtype=np.float32)
        for vt in range(VT_PER_CORE):
            base = 128.0 * seq[vt]
            for k in range(KB):
                t = _win_lo(vt) + k
                xr[:, vt * KB + k] = (xpad[t * P : (t + 1) * P] - base).astype(
                    np.float32
                )
        in_maps.append({"hs": hs, "consts": _pack_consts(xr, iota_np)})
        scatter.append(seq)
    return in_maps, scatter


def _shard_mode_a(H, X, order, Xs, counts, starts, K):
    in_maps = []
    scatter = []
    iota_np = _iota_np()
    for c in range(NCORES):
        hs = np.zeros((P, VT_PER_CORE * K * W), dtype=np.float16)
        xr = np.full((P, VT_PER_CORE * K), -1000.0, dtype=np.float32)
        seq = list(range(c * VT_PER_CORE, (c + 1) * VT_PER_CORE))
        for vt, g in enumerate(seq):
            s, e = int(starts[g]), int(starts[g + 1])
            cnt = e - s
            block = np.zeros((K * P, D), dtype=np.float32)
            block[:cnt] = H[order[s:e]]
            hi, lo = _split_f16(block)
            hs[:, vt * K * W : (vt + 1) * K * W] = _tilemajor(hi, lo, K)
            xv = np.full(K * P, -1000.0, dtype=np.float32)
            xv[:cnt] = (Xs[s:e] - g * P).astype(np.float32)
            xr[:, vt * K : (vt + 1) * K] = xv.reshape(K, P).T
        in_maps.append({"hs": hs, "consts": _pack_consts(xr, iota_np)})
        scatter.append(seq)
    return in_maps, scatter


def kernel(H, X_neis, V=V):
    global LAST_EXEC_NS, LAST_RESULTS
    H = np.asarray(H, dtype=np.float32)
    X = np.asarray(X_neis).astype(np.int64)
    assert H.shape == (N, D) and X.shape == (N,)

    order = np.argsort(X, kind="stable")
    Xs = X[order]
    counts = np.bincount(X, minlength=V).reshape(NVT, P).sum(axis=1)
    starts = np.zeros(NVT + 1, dtype=np.int64)
    np.cumsum(counts, out=starts[1:])

    groups = _partition_tiles(counts)
    if groups is not None:
        mode, K = "B", KB
        in_maps, scatter = _shard_mode_b(H, X, order, Xs, counts, starts, groups)
    else:
        mode, K = "A", max(1, int(-(-counts.max() // P)))
        in_maps, scatter = _shard_mode_a(H, X, order, Xs, counts, starts, K)

    key = (mode, K)
    if key not in _PROGRAM_CACHE:
        _PROGRAM_CACHE[key] = _build_program(mode, K)
    nc = _PROGRAM_CACHE[key]

    res = run_bass_kernel_spmd(nc, in_maps, list(range(NCORES)), trace=TRACE)
    LAST_EXEC_NS = res.exec_time_ns
    LAST_RESULTS = res

    full = np.empty((V, D), dtype=np.float32)
    for c in range(NCORES):
        o = res.results[c]["out"]
        for vt, g in enumerate(scatter[c]):
            full[g * P : (g + 1) * P] = o[vt * P : (vt + 1) * P]
    return full
